# revision 10
# baseline (speedup 1.0000x reference)
"""Compact Bilinear Pooling (B=16, C=512, HW=196, OUT=8192) on 8 TRN2 cores.

Math: the reference computes, per batch b,
    cbp = irfft(rfft(p1) * rfft(p2)) * OUT,  p_j = x_hw @ sketch_j,
summed over the 196 spatial positions, then signed-sqrt + L2 normalize.
Since irfft is linear the spatial sum moves inside the transform:
    Rhat_b[f] = sum_hw U1[hw,f] * U2[hw,f],   U_j = rfft(p_j) = x @ A_j,
where row c of A_j is a DFT-phase row selected by the count-sketch integer
index h_j[c] (the +-1 sign folds in as the row sign). One length-8192
irfft per batch (Cooley-Tukey 33x128 then 64x128, as small matmuls),
signed-sqrt, L2 normalize.

Kernel structure: frequency bins live on PSUM partitions. For each 128-bin
chunk q the table block is the STATIONARY operand and x (both per-core
batches side by side, [128c, 392]) is the MOVING operand. Precision is
fp16 hi/lo (U = xh@Ah + xh@Al + xl@Ah, fp32 PSUM accumulation), which
measured 1.2e-4 end-to-end error on hardware. The Hadamard product and
hw-reduction run on DVE per chunk, writing one spectrum column per
(batch, re/im) into SBUF accumulators [128, 33].

All sketch-derived tables and constants are baked into the NEFF as Const
data (DMA'd to HBM once at model-load time), so the only per-run inputs
are x's fp16 hi part and an e3m4 fp8 lo part (3 bytes per element, exact
to ~2^-16); the output returns as fp16. The program is compiled per
sketch pair (cached by hash).

Sharding: data-parallel over batch, 2 batches per core, no collectives.
"""

import numpy as np

B, C, HW, N = 16, 512, 196, 8192
NF = N // 2 + 1          # 4097 rfft bins
CHUNK = 128              # frequency bins per chunk (PSUM partition dim)
NCHUNK = 33              # 33*128 = 4224 >= 4097
NCORES = 8
BPC = B // NCORES        # batches per core
MW = BPC * HW            # moving columns (both batches): 392
EPS_SQRT = 1e-5
EPS_NORM = 1e-12

_COMPILED = {}


def _build_tables(sketch1, sketch2):
    """T[q, kc, c, 1024]: cols 0:512 = fp16-hi blocks [re1|im1|re2|im2],
    cols 512:1024 = fp16-lo blocks, for bins f = 128q + p."""
    f = np.arange(NCHUNK * CHUNK, dtype=np.int64)
    valid = f < NF

    def tab(sk):
        sk = np.asarray(sk)
        h = np.abs(sk).argmax(axis=1).astype(np.int64)
        s = sk[np.arange(C), h].astype(np.float64)
        ph = (h[:, None] * f[None, :]) % N          # [C, 4224]
        ang = 2.0 * np.pi * ph / N
        re = np.where(valid[None, :], s[:, None] * np.cos(ang), 0.0)
        im = np.where(valid[None, :], -s[:, None] * np.sin(ang), 0.0)
        return re, im                                # [C, 4224] each

    r1, i1 = tab(sketch1)
    r2, i2 = tab(sketch2)

    def blk(a):  # [C, 4224] -> [NCHUNK, 4, 128, 128]
        return a.reshape(4, 128, NCHUNK, CHUNK).transpose(2, 0, 1, 3)

    def pack(r1_, i1_, r2_, i2_):
        return np.concatenate([blk(r1_), blk(i1_), blk(r2_), blk(i2_)],
                              axis=3)

    hi = [a.astype(np.float16).astype(np.float64) for a in (r1, i1, r2, i2)]
    lo = [(a - h).astype(np.float16) for a, h in zip((r1, i1, r2, i2), hi)]
    Ah = pack(*[h.astype(np.float16) for h in hi])
    Al = pack(*lo)
    return np.ascontiguousarray(
        np.concatenate([Ah, Al], axis=3).astype(np.float16))


def _build_consts():
    q = np.arange(NCHUNK)
    m = np.arange(64)
    p = np.arange(128)
    j = np.arange(128)
    E1 = 2 * np.pi * np.outer(q, m) / 64.0          # [33, 64]
    TW = 2 * np.pi * np.outer(m, p) / float(N)      # [64, 128]
    E2 = 2 * np.pi * np.outer(p, j) / 128.0         # [128, 128]
    c = {
        "e1c": np.cos(E1), "e1s": np.sin(E1), "e1sn": -np.sin(E1),
        "twc": np.cos(TW), "tws": np.sin(TW),
        "e2c": np.cos(E2), "e2sn": -np.sin(E2),
        "i128": np.eye(128), "i64": np.eye(64),
        "ones_col64": np.ones((64, 1)),
        "ones_1x64": np.ones((1, 64)),
        "mones_1x64": np.full((1, 64), -1.0),
        "malt_1x64": (-((-1.0) ** np.arange(64))).reshape(1, 64),
    }
    return {k: np.ascontiguousarray(v.astype(np.float32)) for k, v in c.items()}


def _build_program(A, consts):
    import concourse.mybir as mybir
    import concourse.tile as tile
    from concourse import bacc

    f32 = mybir.dt.float32
    f16 = mybir.dt.float16
    f8 = mybir.dt.float8e3
    AF = mybir.ActivationFunctionType
    OP = mybir.AluOpType
    AX = mybir.AxisListType

    nc = bacc.Bacc("TRN2", target_bir_lowering=False, debug=False,
                   num_devices=NCORES)

    # x ships as fp16 hi + e3m4(512*lo): 3 bytes/element
    xh_in = nc.dram_tensor("xh", [4, 128, MW], f16, kind="ExternalInput").ap()
    xl_in = nc.dram_tensor("xl8", [4, 128, MW], f8, kind="ExternalInput").ap()
    atab = nc.inline_tensor(A, name="atab").ap()        # [33, 4, 128, 1024]
    ct_d = {k: nc.inline_tensor(v, name=k).ap() for k, v in consts.items()}
    out = nc.dram_tensor("out", [BPC, 64, 128], f16, kind="ExternalOutput").ap()

    with tile.TileContext(nc) as tc:
        with (
            tc.tile_pool(name="xpool", bufs=1) as xpool,
            tc.tile_pool(name="cpool", bufs=1) as cpool,
            tc.tile_pool(name="spool", bufs=1) as spool,
        ):
            # x to SBUF: fp16 hi directly, e3m4 lo descaled to fp16
            xh, xl = [], []
            for kc in range(4):
                th = xpool.tile([128, MW], f16, tag=f"xh{kc}", name=f"xh{kc}")
                nc.sync.dma_start(th[:], xh_in[kc])
                t8 = xpool.tile([128, MW], f8, tag=f"x8{kc}", name=f"x8{kc}")
                nc.sync.dma_start(t8[:], xl_in[kc])
                tl = xpool.tile([128, MW], f16, tag=f"xl{kc}", name=f"xl{kc}")
                nc.scalar.activation(tl[:], t8[:], AF.Copy, scale=1.0 / 512.0)
                xh.append(th)
                xl.append(tl)
            # consts to SBUF
            ct = {}
            for k in consts:
                t = cpool.tile(list(ct_d[k].shape), f32, tag=k, name=k)
                nc.sync.dma_start(t[:], ct_d[k][:])
                ct[k] = t
            eps_b = cpool.tile([64, 1], f32, tag="eps_b", name="eps_b")
            nc.gpsimd.memset(eps_b[:], EPS_SQRT)
            eps_n = cpool.tile([1, 1], f32, tag="eps_n", name="eps_n")
            nc.gpsimd.memset(eps_n[:], float(N) * EPS_SQRT)
            # spectrum accumulators [128 p, 33 q] per (batch, re/im)
            S = [[spool.tile([128, NCHUNK], f32, tag=f"S{b}{ri}",
                             name=f"S{b}{ri}")
                  for ri in range(2)] for b in range(BPC)]

            # ---- main loop: chunk q covers bins f = 128q + p ----
            with (
                tc.tile_pool(name="apool", bufs=2) as apool,
                tc.tile_pool(name="hpool", bufs=2) as hpool,
                tc.tile_pool(name="upsum", bufs=2, space="PSUM") as upsum,
            ):
                for q in range(NCHUNK):
                    ach = apool.tile([128, 4, 1024], f16, tag="ach",
                                     name="ach")
                    nc.sync.dma_start(ach[:], atab[q].transpose([1, 0, 2]))
                    # u[blk], blk in (re1, im1, re2, im2): [128 p, 392]
                    ups = [upsum.tile([128, MW], f32, tag=f"u{blk}",
                                      name=f"u{blk}")
                           for blk in range(4)]
                    # U = xh @ Ah + xh @ Al + xl @ Ah  (fp32 accumulate)
                    passes = [(xh, 0), (xh, 512), (xl, 0)]
                    for pi, (xop, toff) in enumerate(passes):
                        for kc in range(4):
                            for blk in range(4):
                                sl = slice(toff + blk * 128,
                                           toff + blk * 128 + 128)
                                nc.tensor.matmul(
                                    ups[blk][:], ach[:, kc, sl], xop[kc][:],
                                    start=(pi == 0 and kc == 0),
                                    stop=(pi == 2 and kc == 3))
                    # stage u2 in SBUF (DVE reads at most one PSUM operand)
                    u2r = hpool.tile([128, MW], f32, tag="u2r", name="u2r")
                    u2i = hpool.tile([128, MW], f32, tag="u2i", name="u2i")
                    nc.scalar.copy(u2r[:], ups[2][:])
                    nc.scalar.copy(u2i[:], ups[3][:])
                    # Hadamard + hw-reduce into spectrum columns
                    t1 = hpool.tile([128, MW], f32, tag="t1", name="t1")
                    t2 = hpool.tile([128, MW], f32, tag="t2", name="t2")
                    dre = hpool.tile([128, MW], f32, tag="dre", name="dre")
                    dim = hpool.tile([128, MW], f32, tag="dim", name="dim")
                    nc.vector.tensor_tensor(t1[:], ups[0][:], u2r[:],
                                            op=OP.mult)
                    nc.vector.tensor_tensor(t2[:], ups[1][:], u2i[:],
                                            op=OP.mult)
                    nc.vector.tensor_tensor(dre[:], t1[:], t2[:],
                                            op=OP.subtract)
                    nc.vector.tensor_tensor(t1[:], ups[0][:], u2i[:],
                                            op=OP.mult)
                    nc.vector.tensor_tensor(t2[:], ups[1][:], u2r[:],
                                            op=OP.mult)
                    nc.vector.tensor_tensor(dim[:], t1[:], t2[:], op=OP.add)
                    for b in range(BPC):
                        sl = slice(b * HW, (b + 1) * HW)
                        nc.vector.reduce_sum(S[b][0][:, q:q + 1], dre[:, sl],
                                             axis=AX.X)
                        nc.vector.reduce_sum(S[b][1][:, q:q + 1], dim[:, sl],
                                             axis=AX.X)

            # ---- per batch: irfft (CT 33x128 then 64x128) + tail ----
            with (
                tc.tile_pool(name="wpool", bufs=2) as wpool,
                tc.tile_pool(name="tpsum", bufs=1, space="PSUM") as tpsum,
            ):
                for b in range(BPC):
                    # transpose spectrum [128, 33] -> [33, 128]
                    stp = tpsum.tile([NCHUNK, 256], f32, tag="stp",
                                     name="stp")
                    st = [wpool.tile([NCHUNK, 128], f32, tag=f"st{ri}",
                                     name=f"st{ri}") for ri in range(2)]
                    for ri in range(2):
                        nc.tensor.transpose(stp[:, 128 * ri:128 * (ri + 1)],
                                            S[b][ri][:], ct["i128"][:])
                        nc.scalar.copy(st[ri][:],
                                       stp[:, 128 * ri:128 * (ri + 1)])
                    # stage 1: T[m, p] = sum_q S[q, p] e^{2 pi i m q / 64}
                    tps = tpsum.tile([64, 256], f32, tag="tps", name="tps")
                    tre_ps = tps[:, 0:128]
                    tim_ps = tps[:, 128:256]
                    nc.tensor.matmul(tre_ps, ct["e1c"][:], st[0][:],
                                     start=True, stop=False)
                    nc.tensor.matmul(tre_ps, ct["e1sn"][:], st[1][:],
                                     start=False, stop=True)
                    nc.tensor.matmul(tim_ps, ct["e1s"][:], st[0][:],
                                     start=True, stop=False)
                    nc.tensor.matmul(tim_ps, ct["e1c"][:], st[1][:],
                                     start=False, stop=True)
                    # twiddle by e^{2 pi i m p / 8192}
                    w1 = wpool.tile([64, 128], f32, tag="w1", name="w1")
                    w2 = wpool.tile([64, 128], f32, tag="w2", name="w2")
                    tpr = wpool.tile([64, 128], f32, tag="tpr", name="tpr")
                    tpi = wpool.tile([64, 128], f32, tag="tpi", name="tpi")
                    nc.vector.tensor_tensor(w1[:], tre_ps, ct["twc"][:],
                                            op=OP.mult)
                    nc.vector.tensor_tensor(w2[:], tim_ps, ct["tws"][:],
                                            op=OP.mult)
                    nc.vector.tensor_tensor(tpr[:], w1[:], w2[:],
                                            op=OP.subtract)
                    nc.vector.tensor_tensor(w1[:], tre_ps, ct["tws"][:],
                                            op=OP.mult)
                    nc.vector.tensor_tensor(w2[:], tim_ps, ct["twc"][:],
                                            op=OP.mult)
                    nc.vector.tensor_tensor(tpi[:], w1[:], w2[:], op=OP.add)
                    # transpose T' [64, 128] -> [128, 64]
                    ttp = tpsum.tile([128, 128], f32, tag="ttp", name="ttp")
                    tt = [wpool.tile([128, 64], f32, tag=f"tt{ri}",
                                     name=f"tt{ri}") for ri in range(2)]
                    nc.tensor.transpose(ttp[:, 0:64], tpr[:], ct["i64"][:])
                    nc.tensor.transpose(ttp[:, 64:128], tpi[:], ct["i64"][:])
                    nc.scalar.copy(tt[0][:], ttp[:, 0:64])
                    nc.scalar.copy(tt[1][:], ttp[:, 64:128])
                    # corrections c[m] = -Rhat[0] - (-1)^m Rhat[4096]
                    sml = tpsum.tile([64, 4], f32, tag="sml", name="sml")
                    cps = sml[:, 0:1]
                    nc.tensor.matmul(cps, ct["mones_1x64"][:],
                                     S[b][0][0:1, 0:1], start=True, stop=False)
                    nc.tensor.matmul(cps, ct["malt_1x64"][:],
                                     S[b][0][0:1, 32:33], start=False,
                                     stop=True)
                    c_sb = wpool.tile([64, 1], f32, tag="c_sb", name="c_sb")
                    nc.scalar.copy(c_sb[:], cps)
                    # stage 2: X[m, j] = sum_p T're e2c - T'im e2s
                    xps = tpsum.tile([64, 128], f32, tag="xps", name="xps")
                    nc.tensor.matmul(xps[:], tt[0][:], ct["e2c"][:],
                                     start=True, stop=False)
                    nc.tensor.matmul(xps[:], tt[1][:], ct["e2sn"][:],
                                     start=False, stop=True)
                    # Z = 2*X + c
                    zeff = wpool.tile([64, 128], f32, tag="zeff", name="zeff")
                    nc.vector.tensor_scalar(zeff[:], xps[:], 2.0, c_sb[:, 0:1],
                                            op0=OP.mult, op1=OP.add)
                    # tail: signed sqrt + L2 normalize
                    absz = wpool.tile([64, 128], f32, tag="absz", name="absz")
                    nc.scalar.activation(absz[:], zeff[:], AF.Abs)
                    sq = wpool.tile([64, 128], f32, tag="sq", name="sq")
                    nc.scalar.activation(sq[:], absz[:], AF.Sqrt, bias=eps_b[:])
                    sgn = wpool.tile([64, 128], f32, tag="sgn", name="sgn")
                    nc.scalar.activation(sgn[:], zeff[:], AF.Sign)
                    ssq = wpool.tile([64, 128], f32, tag="ssq", name="ssq")
                    nc.vector.tensor_tensor(ssq[:], sq[:], sgn[:], op=OP.mult)
                    rs = wpool.tile([64, 1], f32, tag="rs", name="rs")
                    nc.vector.reduce_sum(rs[:], zeff[:], axis=AX.X,
                                         apply_absolute_value=True)
                    tot = sml[0:1, 1:2]
                    nc.tensor.matmul(tot, rs[:], ct["ones_col64"][:],
                                     start=True, stop=True)
                    nrm = wpool.tile([1, 1], f32, tag="nrm", name="nrm")
                    nc.scalar.activation(nrm[:], tot, AF.Sqrt,
                                         bias=eps_n[0:1, :])
                    nc.vector.tensor_scalar_max(nrm[:], nrm[:], EPS_NORM)
                    nc.vector.reciprocal(nrm[:], nrm[:])
                    nrmb = sml[:, 2:3]
                    nc.tensor.matmul(nrmb, ct["ones_1x64"][:], nrm[:],
                                     start=True, stop=True)
                    nrmb_s = wpool.tile([64, 1], f32, tag="nrmb_s",
                                        name="nrmb_s")
                    nc.scalar.copy(nrmb_s[:], nrmb)
                    fin = wpool.tile([64, 128], f16, tag="fin", name="fin")
                    nc.vector.tensor_scalar_mul(fin[:], ssq[:], nrmb_s[:])
                    nc.sync.dma_start(out[b], fin[:])

    nc.compile()
    return nc


def _get_program(sketch1, sketch2):
    import hashlib
    key = hashlib.sha1(np.asarray(sketch1).tobytes()
                       + np.asarray(sketch2).tobytes()).hexdigest()
    if _COMPILED.get("key") != key:
        A = _build_tables(sketch1, sketch2)
        _COMPILED["nc"] = _build_program(A, _build_consts())
        _COMPILED["key"] = key
    return _COMPILED["nc"]


def make_in_maps(x, sketch1, sketch2):
    import ml_dtypes
    x = np.ascontiguousarray(np.asarray(x), dtype=np.float32)
    xs = x.reshape(B, C, HW)
    in_maps = []
    for i in range(NCORES):
        # [BPC, C, HW] -> [4 kc, 128, BPC*HW] with cols = b*HW + hw
        xc = xs[i * BPC:(i + 1) * BPC]               # [BPC, C, HW]
        xc = xc.transpose(1, 0, 2).reshape(C, MW)    # [C, BPC*HW]
        xc = xc.reshape(4, 128, MW)
        xh = xc.astype(np.float16)
        xl8 = ((xc - xh.astype(np.float32)) * 512.0).astype(
            ml_dtypes.float8_e3m4)
        in_maps.append({"xh": np.ascontiguousarray(xh),
                        "xl8": np.ascontiguousarray(xl8)})
    return in_maps


def unshard_out(results):
    outs = np.empty((B, N), dtype=np.float32)
    for i in range(NCORES):
        z = results[i]["out"]  # [BPC, 64 m, 128 j] f16; X[m + 64 j] = z[b, m, j]
        for b in range(BPC):
            outs[i * BPC + b] = np.ascontiguousarray(
                z[b].T).reshape(-1).astype(np.float32)
    return outs


def kernel(x, sketch1, sketch2):
    from concourse.bass_utils import run_bass_kernel_spmd

    in_maps = make_in_maps(x, sketch1, sketch2)
    nc = _get_program(sketch1, sketch2)
    res = run_bass_kernel_spmd(nc, in_maps, core_ids=list(range(NCORES)))
    return unshard_out(res.results)


# revision 11
# speedup vs baseline: 1.3209x; 1.3209x over previous
"""Compact Bilinear Pooling (B=16, C=512, HW=196, OUT=8192) on 8 TRN2 cores.

Math: the reference computes, per batch b,
    cbp = irfft(rfft(p1) * rfft(p2)) * OUT,  p_j = x_hw @ sketch_j,
summed over the 196 spatial positions, then signed-sqrt + L2 normalize.
Since irfft is linear the spatial sum moves inside the transform:
    Rhat_b[f] = sum_hw U1[hw,f] * U2[hw,f],   U_j = rfft(p_j) = x @ A_j,
where row c of A_j is a DFT-phase row selected by the count-sketch integer
index h_j[c] (the +-1 sign folds in as the row sign). One length-8192
irfft per batch (Cooley-Tukey 33x128 then 64x128, as small matmuls),
signed-sqrt, L2 normalize.

Kernel structure: frequency bins live on PSUM partitions. For each 128-bin
chunk q the table block is the STATIONARY operand and x (both per-core
batches side by side, [128c, 392]) is the MOVING operand. Precision is
fp16 hi/lo (U = xh@Ah + xh@Al + xl@Ah, fp32 PSUM accumulation), which
measured 1.2e-4 end-to-end error on hardware. The Hadamard product and
hw-reduction run on DVE per chunk, writing one spectrum column per
(batch, re/im) into SBUF accumulators [128, 33].

All sketch-derived tables and constants are baked into the NEFF as Const
data (DMA'd to HBM once at model-load time), so the only per-run inputs
are x's fp16 hi part and an e3m4 fp8 lo part (3 bytes per element, exact
to ~2^-16); the output returns as fp16. The program is compiled per
sketch pair (cached by hash).

Sharding: data-parallel over batch, 2 batches per core, no collectives.
"""

import numpy as np

B, C, HW, N = 16, 512, 196, 8192
NF = N // 2 + 1          # 4097 rfft bins
CHUNK = 128              # frequency bins per chunk (PSUM partition dim)
NCHUNK = 33              # 33*128 = 4224 >= 4097
NCORES = 8
BPC = B // NCORES        # batches per core
MW = BPC * HW            # moving columns (both batches): 392
EPS_SQRT = 1e-5
EPS_NORM = 1e-12

_COMPILED = {}


def _build_tables(sketch1, sketch2):
    """T[q, kc, c, 1024]: cols 0:512 = fp16-hi blocks [re1|im1|re2|im2],
    cols 512:1024 = fp16-lo blocks, for bins f = 128q + p."""
    f = np.arange(NCHUNK * CHUNK, dtype=np.int64)
    valid = f < NF

    def tab(sk):
        sk = np.asarray(sk)
        h = np.abs(sk).argmax(axis=1).astype(np.int64)
        s = sk[np.arange(C), h].astype(np.float64)
        ph = (h[:, None] * f[None, :]) % N          # [C, 4224]
        ang = 2.0 * np.pi * ph / N
        re = np.where(valid[None, :], s[:, None] * np.cos(ang), 0.0)
        im = np.where(valid[None, :], -s[:, None] * np.sin(ang), 0.0)
        return re, im                                # [C, 4224] each

    r1, i1 = tab(sketch1)
    r2, i2 = tab(sketch2)

    def blk(a):  # [C, 4224] -> [NCHUNK, 4, 128, 128]
        return a.reshape(4, 128, NCHUNK, CHUNK).transpose(2, 0, 1, 3)

    def pack(r1_, i1_, r2_, i2_):
        return np.concatenate([blk(r1_), blk(i1_), blk(r2_), blk(i2_)],
                              axis=3)

    hi = [a.astype(np.float16).astype(np.float64) for a in (r1, i1, r2, i2)]
    lo = [(a - h).astype(np.float16) for a, h in zip((r1, i1, r2, i2), hi)]
    Ah = pack(*[h.astype(np.float16) for h in hi])
    Al = pack(*lo)
    return np.ascontiguousarray(
        np.concatenate([Ah, Al], axis=3).astype(np.float16))


def _build_consts():
    q = np.arange(NCHUNK)
    m = np.arange(64)
    p = np.arange(128)
    j = np.arange(128)
    E1 = 2 * np.pi * np.outer(q, m) / 64.0          # [33, 64]
    TW = 2 * np.pi * np.outer(m, p) / float(N)      # [64, 128]
    E2 = 2 * np.pi * np.outer(p, j) / 128.0         # [128, 128]
    c = {
        "e1c": np.cos(E1), "e1s": np.sin(E1), "e1sn": -np.sin(E1),
        "twc": np.cos(TW), "tws": np.sin(TW),
        "e2c": np.cos(E2), "e2sn": -np.sin(E2),
        "i128": np.eye(128), "i64": np.eye(64),
        "ones_col64": np.ones((64, 1)),
        "ones_1x64": np.ones((1, 64)),
        "mones_1x64": np.full((1, 64), -1.0),
        "malt_1x64": (-((-1.0) ** np.arange(64))).reshape(1, 64),
    }
    return {k: np.ascontiguousarray(v.astype(np.float32)) for k, v in c.items()}


def _build_program(A, consts):
    import concourse.mybir as mybir
    import concourse.tile as tile
    from concourse import bacc

    f32 = mybir.dt.float32
    f16 = mybir.dt.float16
    f8 = mybir.dt.float8e3
    AF = mybir.ActivationFunctionType
    OP = mybir.AluOpType
    AX = mybir.AxisListType

    nc = bacc.Bacc("TRN2", target_bir_lowering=False, debug=False,
                   num_devices=NCORES)

    # x ships as fp16 hi + e3m4(512*lo): 3 bytes/element
    xh_in = nc.dram_tensor("xh", [4, 128, MW], f16, kind="ExternalInput").ap()
    xl_in = nc.dram_tensor("xl8", [4, 128, MW], f8, kind="ExternalInput").ap()
    atab = nc.inline_tensor(A, name="atab").ap()        # [33, 4, 128, 1024]
    ct_d = {k: nc.inline_tensor(v, name=k).ap() for k, v in consts.items()}
    out = nc.dram_tensor("out", [BPC, 64, 128], f16, kind="ExternalOutput").ap()

    with tile.TileContext(nc) as tc:
        with (
            tc.tile_pool(name="xpool", bufs=1) as xpool,
            tc.tile_pool(name="cpool", bufs=1) as cpool,
            tc.tile_pool(name="spool", bufs=1) as spool,
        ):
            # x to SBUF: fp16 hi directly, e3m4 lo descaled to fp16
            xh, xl = [], []
            for kc in range(4):
                th = xpool.tile([128, MW], f16, tag=f"xh{kc}", name=f"xh{kc}")
                nc.sync.dma_start(th[:], xh_in[kc])
                t8 = xpool.tile([128, MW], f8, tag=f"x8{kc}", name=f"x8{kc}")
                nc.sync.dma_start(t8[:], xl_in[kc])
                tl = xpool.tile([128, MW], f16, tag=f"xl{kc}", name=f"xl{kc}")
                nc.scalar.activation(tl[:], t8[:], AF.Copy, scale=1.0 / 512.0)
                xh.append(th)
                xl.append(tl)
            # consts to SBUF
            ct = {}
            for k in consts:
                t = cpool.tile(list(ct_d[k].shape), f32, tag=k, name=k)
                nc.sync.dma_start(t[:], ct_d[k][:])
                ct[k] = t
            eps_b = cpool.tile([64, 1], f32, tag="eps_b", name="eps_b")
            nc.gpsimd.memset(eps_b[:], EPS_SQRT)
            eps_n = cpool.tile([1, 1], f32, tag="eps_n", name="eps_n")
            nc.gpsimd.memset(eps_n[:], float(N) * EPS_SQRT)
            # spectrum accumulators [128 p, 33 q] per (batch, re/im)
            S = [[spool.tile([128, NCHUNK], f32, tag=f"S{b}{ri}",
                             name=f"S{b}{ri}")
                  for ri in range(2)] for b in range(BPC)]

            # ---- main loop: chunk q covers bins f = 128q + p ----
            with (
                tc.tile_pool(name="apool", bufs=2) as apool,
                tc.tile_pool(name="hpool", bufs=2) as hpool,
                tc.tile_pool(name="upsum", bufs=2, space="PSUM") as upsum,
            ):
                for q in range(NCHUNK):
                    ach = apool.tile([128, 4, 1024], f16, tag="ach",
                                     name="ach")
                    # alternate the two HWDGE rings (SP / ACT) so table
                    # loads don't serialize on one queue
                    dma_eng = nc.sync if q % 2 == 0 else nc.scalar
                    dma_eng.dma_start(ach[:], atab[q].transpose([1, 0, 2]))
                    # u[blk], blk in (re1, im1, re2, im2): [128 p, 392]
                    ups = [upsum.tile([128, MW], f32, tag=f"u{blk}",
                                      name=f"u{blk}")
                           for blk in range(4)]
                    # U = xh @ Ah + xh @ Al + xl @ Ah  (fp32 accumulate)
                    passes = [(xh, 0), (xh, 512), (xl, 0)]
                    for pi, (xop, toff) in enumerate(passes):
                        for kc in range(4):
                            for blk in range(4):
                                sl = slice(toff + blk * 128,
                                           toff + blk * 128 + 128)
                                nc.tensor.matmul(
                                    ups[blk][:], ach[:, kc, sl], xop[kc][:],
                                    start=(pi == 0 and kc == 0),
                                    stop=(pi == 2 and kc == 3))
                    # stage u2 in SBUF (DVE reads at most one PSUM operand)
                    u2r = hpool.tile([128, MW], f32, tag="u2r", name="u2r")
                    u2i = hpool.tile([128, MW], f32, tag="u2i", name="u2i")
                    nc.scalar.copy(u2r[:], ups[2][:])
                    nc.scalar.copy(u2i[:], ups[3][:])
                    # Hadamard + hw-reduce into spectrum columns
                    t1 = hpool.tile([128, MW], f32, tag="t1", name="t1")
                    t2 = hpool.tile([128, MW], f32, tag="t2", name="t2")
                    dre = hpool.tile([128, MW], f32, tag="dre", name="dre")
                    dim = hpool.tile([128, MW], f32, tag="dim", name="dim")
                    nc.vector.tensor_tensor(t1[:], ups[0][:], u2r[:],
                                            op=OP.mult)
                    nc.vector.tensor_tensor(t2[:], ups[1][:], u2i[:],
                                            op=OP.mult)
                    nc.vector.tensor_tensor(dre[:], t1[:], t2[:],
                                            op=OP.subtract)
                    nc.vector.tensor_tensor(t1[:], ups[0][:], u2i[:],
                                            op=OP.mult)
                    nc.vector.tensor_tensor(t2[:], ups[1][:], u2r[:],
                                            op=OP.mult)
                    nc.vector.tensor_tensor(dim[:], t1[:], t2[:], op=OP.add)
                    for b in range(BPC):
                        sl = slice(b * HW, (b + 1) * HW)
                        nc.vector.reduce_sum(S[b][0][:, q:q + 1], dre[:, sl],
                                             axis=AX.X)
                        nc.vector.reduce_sum(S[b][1][:, q:q + 1], dim[:, sl],
                                             axis=AX.X)

            # ---- per batch: irfft (CT 33x128 then 64x128) + tail ----
            with (
                tc.tile_pool(name="wpool", bufs=2) as wpool,
                tc.tile_pool(name="tpsum", bufs=1, space="PSUM") as tpsum,
            ):
                for b in range(BPC):
                    # transpose spectrum [128, 33] -> [33, 128]
                    stp = tpsum.tile([NCHUNK, 256], f32, tag="stp",
                                     name="stp")
                    st = [wpool.tile([NCHUNK, 128], f32, tag=f"st{ri}",
                                     name=f"st{ri}") for ri in range(2)]
                    for ri in range(2):
                        nc.tensor.transpose(stp[:, 128 * ri:128 * (ri + 1)],
                                            S[b][ri][:], ct["i128"][:])
                        nc.scalar.copy(st[ri][:],
                                       stp[:, 128 * ri:128 * (ri + 1)])
                    # stage 1: T[m, p] = sum_q S[q, p] e^{2 pi i m q / 64}
                    tps = tpsum.tile([64, 256], f32, tag="tps", name="tps")
                    tre_ps = tps[:, 0:128]
                    tim_ps = tps[:, 128:256]
                    nc.tensor.matmul(tre_ps, ct["e1c"][:], st[0][:],
                                     start=True, stop=False)
                    nc.tensor.matmul(tre_ps, ct["e1sn"][:], st[1][:],
                                     start=False, stop=True)
                    nc.tensor.matmul(tim_ps, ct["e1s"][:], st[0][:],
                                     start=True, stop=False)
                    nc.tensor.matmul(tim_ps, ct["e1c"][:], st[1][:],
                                     start=False, stop=True)
                    # twiddle by e^{2 pi i m p / 8192}
                    w1 = wpool.tile([64, 128], f32, tag="w1", name="w1")
                    w2 = wpool.tile([64, 128], f32, tag="w2", name="w2")
                    tpr = wpool.tile([64, 128], f32, tag="tpr", name="tpr")
                    tpi = wpool.tile([64, 128], f32, tag="tpi", name="tpi")
                    nc.vector.tensor_tensor(w1[:], tre_ps, ct["twc"][:],
                                            op=OP.mult)
                    nc.vector.tensor_tensor(w2[:], tim_ps, ct["tws"][:],
                                            op=OP.mult)
                    nc.vector.tensor_tensor(tpr[:], w1[:], w2[:],
                                            op=OP.subtract)
                    nc.vector.tensor_tensor(w1[:], tre_ps, ct["tws"][:],
                                            op=OP.mult)
                    nc.vector.tensor_tensor(w2[:], tim_ps, ct["twc"][:],
                                            op=OP.mult)
                    nc.vector.tensor_tensor(tpi[:], w1[:], w2[:], op=OP.add)
                    # transpose T' [64, 128] -> [128, 64]
                    ttp = tpsum.tile([128, 128], f32, tag="ttp", name="ttp")
                    tt = [wpool.tile([128, 64], f32, tag=f"tt{ri}",
                                     name=f"tt{ri}") for ri in range(2)]
                    nc.tensor.transpose(ttp[:, 0:64], tpr[:], ct["i64"][:])
                    nc.tensor.transpose(ttp[:, 64:128], tpi[:], ct["i64"][:])
                    nc.scalar.copy(tt[0][:], ttp[:, 0:64])
                    nc.scalar.copy(tt[1][:], ttp[:, 64:128])
                    # corrections c[m] = -Rhat[0] - (-1)^m Rhat[4096]
                    sml = tpsum.tile([64, 4], f32, tag="sml", name="sml")
                    cps = sml[:, 0:1]
                    nc.tensor.matmul(cps, ct["mones_1x64"][:],
                                     S[b][0][0:1, 0:1], start=True, stop=False)
                    nc.tensor.matmul(cps, ct["malt_1x64"][:],
                                     S[b][0][0:1, 32:33], start=False,
                                     stop=True)
                    c_sb = wpool.tile([64, 1], f32, tag="c_sb", name="c_sb")
                    nc.scalar.copy(c_sb[:], cps)
                    # stage 2: X[m, j] = sum_p T're e2c - T'im e2s
                    xps = tpsum.tile([64, 128], f32, tag="xps", name="xps")
                    nc.tensor.matmul(xps[:], tt[0][:], ct["e2c"][:],
                                     start=True, stop=False)
                    nc.tensor.matmul(xps[:], tt[1][:], ct["e2sn"][:],
                                     start=False, stop=True)
                    # Z = 2*X + c
                    zeff = wpool.tile([64, 128], f32, tag="zeff", name="zeff")
                    nc.vector.tensor_scalar(zeff[:], xps[:], 2.0, c_sb[:, 0:1],
                                            op0=OP.mult, op1=OP.add)
                    # tail: signed sqrt + L2 normalize
                    absz = wpool.tile([64, 128], f32, tag="absz", name="absz")
                    nc.scalar.activation(absz[:], zeff[:], AF.Abs)
                    sq = wpool.tile([64, 128], f32, tag="sq", name="sq")
                    nc.scalar.activation(sq[:], absz[:], AF.Sqrt, bias=eps_b[:])
                    sgn = wpool.tile([64, 128], f32, tag="sgn", name="sgn")
                    nc.scalar.activation(sgn[:], zeff[:], AF.Sign)
                    ssq = wpool.tile([64, 128], f32, tag="ssq", name="ssq")
                    nc.vector.tensor_tensor(ssq[:], sq[:], sgn[:], op=OP.mult)
                    rs = wpool.tile([64, 1], f32, tag="rs", name="rs")
                    nc.vector.reduce_sum(rs[:], zeff[:], axis=AX.X,
                                         apply_absolute_value=True)
                    tot = sml[0:1, 1:2]
                    nc.tensor.matmul(tot, rs[:], ct["ones_col64"][:],
                                     start=True, stop=True)
                    nrm = wpool.tile([1, 1], f32, tag="nrm", name="nrm")
                    nc.scalar.activation(nrm[:], tot, AF.Sqrt,
                                         bias=eps_n[0:1, :])
                    nc.vector.tensor_scalar_max(nrm[:], nrm[:], EPS_NORM)
                    nc.vector.reciprocal(nrm[:], nrm[:])
                    nrmb = sml[:, 2:3]
                    nc.tensor.matmul(nrmb, ct["ones_1x64"][:], nrm[:],
                                     start=True, stop=True)
                    nrmb_s = wpool.tile([64, 1], f32, tag="nrmb_s",
                                        name="nrmb_s")
                    nc.scalar.copy(nrmb_s[:], nrmb)
                    fin = wpool.tile([64, 128], f16, tag="fin", name="fin")
                    nc.vector.tensor_scalar_mul(fin[:], ssq[:], nrmb_s[:])
                    nc.sync.dma_start(out[b], fin[:])

    nc.compile()
    return nc


def _get_program(sketch1, sketch2):
    import hashlib
    key = hashlib.sha1(np.asarray(sketch1).tobytes()
                       + np.asarray(sketch2).tobytes()).hexdigest()
    if _COMPILED.get("key") != key:
        A = _build_tables(sketch1, sketch2)
        _COMPILED["nc"] = _build_program(A, _build_consts())
        _COMPILED["key"] = key
    return _COMPILED["nc"]


def make_in_maps(x, sketch1, sketch2):
    import ml_dtypes
    x = np.ascontiguousarray(np.asarray(x), dtype=np.float32)
    xs = x.reshape(B, C, HW)
    in_maps = []
    for i in range(NCORES):
        # [BPC, C, HW] -> [4 kc, 128, BPC*HW] with cols = b*HW + hw
        xc = xs[i * BPC:(i + 1) * BPC]               # [BPC, C, HW]
        xc = xc.transpose(1, 0, 2).reshape(C, MW)    # [C, BPC*HW]
        xc = xc.reshape(4, 128, MW)
        xh = xc.astype(np.float16)
        xl8 = ((xc - xh.astype(np.float32)) * 512.0).astype(
            ml_dtypes.float8_e3m4)
        in_maps.append({"xh": np.ascontiguousarray(xh),
                        "xl8": np.ascontiguousarray(xl8)})
    return in_maps


def unshard_out(results):
    outs = np.empty((B, N), dtype=np.float32)
    for i in range(NCORES):
        z = results[i]["out"]  # [BPC, 64 m, 128 j] f16; X[m + 64 j] = z[b, m, j]
        for b in range(BPC):
            outs[i * BPC + b] = np.ascontiguousarray(
                z[b].T).reshape(-1).astype(np.float32)
    return outs


def kernel(x, sketch1, sketch2):
    from concourse.bass_utils import run_bass_kernel_spmd

    in_maps = make_in_maps(x, sketch1, sketch2)
    nc = _get_program(sketch1, sketch2)
    res = run_bass_kernel_spmd(nc, in_maps, core_ids=list(range(NCORES)))
    return unshard_out(res.results)
